# revision 10
# baseline (speedup 1.0000x reference)
"""DeepSeek-style MLA decode attention (batch=8, 128 heads, cache 512) on 8 NeuronCores.

Sharding: tensor-parallel over heads (16 heads/core).
 - q LoRA path sharded over the rank dim (Wq_down cols / Wq_up rows); partial
   q summed+scattered to head owners with a ReduceScatter.
 - Wkv_down replicated (c_kv computed fully on every core).
 - k_cache passed host-pretransposed as [h, b, d, keys]; v_cache as [h, b, keys, d].
 - o_proj input rows sharded by head; partial outputs ReduceScattered over the
   batch dim (core b returns batch b's final row).

Note: the reference's "new token" softmax is over a length-1 axis (== 1.0), so
k_new/Wk_up are dead and the new-token contribution is simply + v_new.
"""

import numpy as np

import concourse.bass as bass
import concourse.mybir as mybir
import concourse.tile as tile
from concourse import bacc
from concourse import bass_utils
from concourse.masks import make_identity

NC_ = 8                      # cores
B = 8                        # batch
H = 128                      # total heads
HP = H // NC_                # 16 heads per core
D = 128                      # head dim
L = 512                      # cache len
HID = 7168
QL = 1536
QLP = QL // NC_              # 192
KVL = 512
NH = HP * D                  # 2048 per-core head cols
SCALE = 1.0 / float(np.sqrt(D))
F32 = mybir.dt.float32


def build_nc():
    nc = bacc.Bacc(
        "TRN2",
        target_bir_lowering=False,
        debug=False,
        enable_asserts=True,
        num_devices=NC_,
    )
    xt = nc.dram_tensor("xt", [HID, B], F32, kind="ExternalInput").ap()
    w_down = nc.dram_tensor("w_down", [HID, QLP + KVL], F32, kind="ExternalInput").ap()
    wq_up = nc.dram_tensor("wq_up", [QLP, H * D], F32, kind="ExternalInput").ap()
    wv_up = nc.dram_tensor("wv_up", [KVL, NH], F32, kind="ExternalInput").ap()
    kt = nc.dram_tensor("kt", [HP, B, D, L], F32, kind="ExternalInput").ap()
    v = nc.dram_tensor("v", [HP, B, L, D], F32, kind="ExternalInput").ap()
    wo = nc.dram_tensor("wo", [NH, HID], F32, kind="ExternalInput").ap()
    o = nc.dram_tensor("o", [1, HID], F32, kind="ExternalOutput").ap()

    rg = [list(range(NC_))]

    with tile.TileContext(nc) as tc:
        with (
            tc.tile_pool(name="const", bufs=1) as constp,
            tc.tile_pool(name="sbuf", bufs=1) as sb,
            tc.tile_pool(name="stage", bufs=2) as stg,
            tc.tile_pool(name="wdown", bufs=3) as wdp,
            tc.tile_pool(name="wqup", bufs=2) as wqp,
            tc.tile_pool(name="ktp", bufs=4) as ktp,
            tc.tile_pool(name="vp", bufs=4) as vp,
            tc.tile_pool(name="wop", bufs=4) as wop,
            tc.tile_pool(name="psmm", bufs=2, space="PSUM") as psmm,
            tc.tile_pool(name="psbig", bufs=2, space="PSUM") as psbig,
            tc.tile_pool(name="psattn", bufs=1, space="PSUM") as psattn,
            tc.tile_pool(name="pstr", bufs=2, space="PSUM") as pstr,
            tc.tile_pool(name="dram", bufs=1, space="DRAM") as dram,
        ):
            ident = constp.tile([128, 128], F32)
            make_identity(nc, ident[:])
            id8 = ident[0:8, 0:8]

            # ---------------- q path: cdown = x @ [Wq_down_c | Wkv_down] ----------------
            xt_sb = constp.tile([128, 56 * B], F32, tag="xt")
            nc.sync.dma_start(
                out=xt_sb[:].rearrange("p (c b) -> p c b", c=56),
                in_=xt.rearrange("(c p) b -> p c b", p=128),
            )
            ps_cd0 = psmm.tile([8, 512], F32, tag="mm")
            ps_cd1 = psmm.tile([8, 512], F32, tag="mm")
            for i in range(56):
                wd_t = wdp.tile([128, QLP + KVL], F32, tag="wd")
                nc.sync.dma_start(out=wd_t[:], in_=w_down[i * 128:(i + 1) * 128, :])
                lhs = xt_sb[:, i * B:(i + 1) * B]
                nc.tensor.matmul(
                    ps_cd0[:8, 0:512], lhs, wd_t[:, 0:512],
                    start=(i == 0), stop=(i == 55),
                )
                nc.tensor.matmul(
                    ps_cd1[:8, 0:192], lhs, wd_t[:, 512:704],
                    start=(i == 0), stop=(i == 55),
                )
            cdown = sb.tile([8, QLP + KVL], F32, tag="cdown")
            nc.vector.tensor_copy(cdown[:, 0:512], ps_cd0[:8, 0:512])
            nc.vector.tensor_copy(cdown[:, 512:704], ps_cd1[:8, 0:192])

            # transposes: cqT [192, 8] (2 chunks), ckvT [512dims -> 4 chunks of [128, 8]]
            ps_cqT = pstr.tile([128, 128], F32, tag="tr")
            nc.tensor.transpose(ps_cqT[0:128, 0:8], cdown[:, 0:128], id8)
            nc.tensor.transpose(ps_cqT[0:64, 8:16], cdown[:, 128:192], id8)
            ps_ckvT = pstr.tile([128, 128], F32, tag="tr")
            for j in range(4):
                nc.tensor.transpose(
                    ps_ckvT[0:128, j * 8:(j + 1) * 8],
                    cdown[:, QLP + j * 128:QLP + (j + 1) * 128],
                    id8,
                )
            cqT = sb.tile([128, 16], F32, tag="cqT")
            nc.vector.tensor_copy(cqT[:, 0:8], ps_cqT[:, 0:8])
            nc.vector.tensor_copy(cqT[0:64, 8:16], ps_cqT[0:64, 8:16])
            ckvT = sb.tile([128, 32], F32, tag="ckvT")
            nc.vector.tensor_copy(ckvT[:, 0:32], ps_ckvT[:, 0:32])

            # ---------------- q_part = cq @ Wq_up_c  (8, 16384) ----------------
            # The 8 col-groups of 2048 are exactly the per-core head groups g;
            # store each to q_bounce[g] for the ReduceScatter.
            q_bounce = dram.tile([NC_ * B, NH], F32, tag="qb")
            for g in range(NC_):
                wqA = wqp.tile([128, 2048], F32, tag="wqA")
                nc.sync.dma_start(
                    out=wqA[:], in_=wq_up[0:128, g * 2048:(g + 1) * 2048]
                )
                wqB = wqp.tile([64, 2048], F32, tag="wqB")
                nc.sync.dma_start(
                    out=wqB[:], in_=wq_up[128:192, g * 2048:(g + 1) * 2048]
                )
                qstage = stg.tile([8, NH], F32, tag="qstage")
                for j in range(4):
                    ps_q = psmm.tile([8, 512], F32, tag="mm")
                    nc.tensor.matmul(
                        ps_q[:8, :], cqT[:, 0:8], wqA[:, j * 512:(j + 1) * 512],
                        start=True, stop=False,
                    )
                    nc.tensor.matmul(
                        ps_q[:8, :], cqT[0:64, 8:16], wqB[:, j * 512:(j + 1) * 512],
                        start=False, stop=True,
                    )
                    nc.vector.tensor_copy(
                        qstage[:, j * 512:(j + 1) * 512], ps_q[:8, :]
                    )
                nc.sync.dma_start(
                    out=q_bounce[g * B:(g + 1) * B, :], in_=qstage[:]
                )
            q_rs = dram.tile([B, NH], F32, tag="qrs")
            nc.gpsimd.collective_compute(
                "ReduceScatter",
                mybir.AluOpType.add,
                replica_groups=rg,
                ins=[q_bounce.opt()],
                outs=[q_rs.opt()],
            )
            qown = sb.tile([8, NH], F32, tag="qown")
            nc.sync.dma_start(out=qown[:], in_=q_rs[:])

            # ---------------- v_new = ckv @ Wv_up_c  (8, 2048) ----------------
            wvup = constp.tile([128, 4 * NH], F32, tag="wvup")
            nc.sync.dma_start(
                out=wvup[:].rearrange("p (c n) -> p c n", c=4),
                in_=wv_up.rearrange("(c p) n -> p c n", p=128),
            )
            vnew = sb.tile([8, NH], F32, tag="vnew")
            for j in range(4):
                ps_v = psmm.tile([8, 512], F32, tag="mm")
                for cc in range(4):
                    nc.tensor.matmul(
                        ps_v[:8, :],
                        ckvT[:, cc * 8:(cc + 1) * 8],
                        wvup[:, cc * NH + j * 512:cc * NH + (j + 1) * 512],
                        start=(cc == 0), stop=(cc == 3),
                    )
                nc.vector.tensor_copy(vnew[:, j * 512:(j + 1) * 512], ps_v[:8, :])

            # qT [128 d, hb] via 16 transposes
            ps_qT = pstr.tile([128, 128], F32, tag="tr")
            for h in range(HP):
                nc.tensor.transpose(
                    ps_qT[0:128, h * 8:(h + 1) * 8],
                    qown[:, h * D:(h + 1) * D],
                    id8,
                )
            qT = sb.tile([128, 128], F32, tag="qT")
            nc.vector.tensor_copy(qT[:], ps_qT[:])

            # ---------------- phase A: scores over k cache ----------------
            kt_flat = kt.rearrange("h b d k -> (h b) d k")
            ps_sT = psbig.tile([128, 512], F32, tag="big")
            for t in range(64):
                kt_t = ktp.tile([128, 1024], F32, tag="kt")
                nc.sync.dma_start(
                    out=kt_t[:].rearrange("d (t k) -> d t k", t=2),
                    in_=kt_flat[2 * t:2 * t + 2].rearrange("t d k -> d t k"),
                )
                for u in range(2):
                    hb = 2 * t + u
                    for cc in range(4):
                        nc.tensor.matmul(
                            ps_sT[:, cc * 128 + hb:cc * 128 + hb + 1],
                            kt_t[:, u * 512 + cc * 128:u * 512 + (cc + 1) * 128],
                            qT[:, hb:hb + 1],
                            start=True, stop=True,
                        )
            sT = sb.tile([128, 512], F32, tag="sT")
            nc.vector.tensor_copy(sT[:], ps_sT[:])

            ps_sc = psbig.tile([128, 512], F32, tag="big")
            for cc in range(4):
                nc.tensor.transpose(
                    ps_sc[:, cc * 128:(cc + 1) * 128],
                    sT[:, cc * 128:(cc + 1) * 128],
                    ident[:],
                )
            probs = sb.tile([128, 512], F32, tag="probs")
            denom = sb.tile([128, 1], F32, tag="denom")
            nc.scalar.activation(
                probs[:], ps_sc[:], mybir.ActivationFunctionType.Exp,
                scale=SCALE, accum_out=denom[:],
            )
            recip = sb.tile([128, 1], F32, tag="recip")
            nc.vector.reciprocal(recip[:], denom[:])
            probsn = sb.tile([128, 512], F32, tag="probsn")
            nc.vector.tensor_scalar_mul(probsn[:], probs[:], recip[:])

            ps_pT = psbig.tile([128, 512], F32, tag="big")
            for cc in range(4):
                nc.tensor.transpose(
                    ps_pT[:, cc * 128:(cc + 1) * 128],
                    probsn[:, cc * 128:(cc + 1) * 128],
                    ident[:],
                )
            probsT = sb.tile([128, 512], F32, tag="probsT")
            nc.vector.tensor_copy(probsT[:], ps_pT[:])

            # ---------------- phase B: attn_T = V^T probs + v_new^T ----------------
            v_flat = v.rearrange("h b l d -> (h b) l d")
            ps_attn = psattn.tile([128, 128], F32, tag="attn")
            for t in range(64):
                v_t = vp.tile([128, 1024], F32, tag="v")
                nc.sync.dma_start(
                    out=v_t[:].rearrange("k (t c d) -> k t c d", t=2, c=4),
                    in_=v_flat[2 * t:2 * t + 2].rearrange(
                        "t (c k) d -> k t c d", c=4
                    ),
                )
                for u in range(2):
                    hb = 2 * t + u
                    for cc in range(4):
                        nc.tensor.matmul(
                            ps_attn[:, hb:hb + 1],
                            v_t[:, u * 512 + cc * 128:u * 512 + (cc + 1) * 128],
                            probsT[:, cc * 128 + hb:cc * 128 + hb + 1],
                            start=(cc == 0), stop=(cc == 3),
                        )

            ps_vT = pstr.tile([128, 128], F32, tag="tr")
            for h in range(HP):
                nc.tensor.transpose(
                    ps_vT[0:128, h * 8:(h + 1) * 8],
                    vnew[:, h * D:(h + 1) * D],
                    id8,
                )
            vnewT = sb.tile([128, 128], F32, tag="vnewT")
            nc.vector.tensor_copy(vnewT[:], ps_vT[:])
            attnT = sb.tile([128, 128], F32, tag="attnT")
            nc.vector.tensor_add(attnT[:], ps_attn[:], vnewT[:])

            # ---------------- phase C: o_part = attn^T @ Wo_c ----------------
            o_bounce = dram.tile([B, HID], F32, tag="ob")
            for n in range(14):
                ps_o = psmm.tile([8, 512], F32, tag="mm")
                for h in range(HP):
                    wo_t = wop.tile([128, 512], F32, tag="wo")
                    nc.sync.dma_start(
                        out=wo_t[:],
                        in_=wo[h * D:(h + 1) * D, n * 512:(n + 1) * 512],
                    )
                    nc.tensor.matmul(
                        ps_o[:8, :],
                        attnT[:, h * 8:(h + 1) * 8],
                        wo_t[:],
                        start=(h == 0), stop=(h == HP - 1),
                    )
                ostage = stg.tile([8, 512], F32, tag="ostage")
                nc.vector.tensor_copy(ostage[:], ps_o[:8, :])
                nc.sync.dma_start(
                    out=o_bounce[:, n * 512:(n + 1) * 512], in_=ostage[:]
                )

            o_rs = dram.tile([1, HID], F32, tag="ors")
            nc.gpsimd.collective_compute(
                "ReduceScatter",
                mybir.AluOpType.add,
                replica_groups=rg,
                ins=[o_bounce.opt()],
                outs=[o_rs.opt()],
            )
            nc.sync.dma_start(out=o[:], in_=o_rs[:])

    nc.compile()
    return nc


_NC_CACHE = None


def _get_nc():
    global _NC_CACHE
    if _NC_CACHE is None:
        _NC_CACHE = build_nc()
    return _NC_CACHE


def make_in_maps(x, k_cache, v_cache, Wq_down, Wq_up, Wkv_down, Wv_up, Wo):
    x2 = np.ascontiguousarray(np.asarray(x, dtype=np.float32).reshape(B, HID).T)
    in_maps = []
    for c in range(NC_):
        hs = slice(c * HP, (c + 1) * HP)
        w_down_c = np.ascontiguousarray(
            np.concatenate(
                [Wq_down[:, c * QLP:(c + 1) * QLP], Wkv_down], axis=1
            ).astype(np.float32)
        )
        wq_up_c = np.ascontiguousarray(Wq_up[c * QLP:(c + 1) * QLP, :], dtype=np.float32)
        wv_up_c = np.ascontiguousarray(
            Wv_up[:, c * HP * D:(c + 1) * HP * D], dtype=np.float32
        )
        wo_c = np.ascontiguousarray(
            Wo[c * HP * D:(c + 1) * HP * D, :], dtype=np.float32
        )
        kt_c = np.ascontiguousarray(
            np.asarray(k_cache, dtype=np.float32)[:, hs].transpose(1, 0, 3, 2)
        )
        v_c = np.ascontiguousarray(
            np.asarray(v_cache, dtype=np.float32)[:, hs].transpose(1, 0, 2, 3)
        )
        in_maps.append(
            {
                "xt": x2,
                "w_down": w_down_c,
                "wq_up": wq_up_c,
                "wv_up": wv_up_c,
                "kt": kt_c,
                "v": v_c,
                "wo": wo_c,
            }
        )
    return in_maps


def kernel(x, k_cache, v_cache, Wq_down, Wq_up, Wkv_down, Wk_up, Wv_up, Wo, **_):
    x = np.asarray(x, dtype=np.float32)
    in_maps = make_in_maps(
        x, np.asarray(k_cache), np.asarray(v_cache),
        np.asarray(Wq_down, dtype=np.float32), np.asarray(Wq_up, dtype=np.float32),
        np.asarray(Wkv_down, dtype=np.float32), np.asarray(Wv_up, dtype=np.float32),
        np.asarray(Wo, dtype=np.float32),
    )
    nc = _get_nc()
    res = bass_utils.run_bass_kernel_spmd(nc, in_maps, core_ids=list(range(NC_)))
    out = np.stack([res.results[b]["o"] for b in range(B)], axis=0)  # (8, 1, 7168)
    return np.ascontiguousarray(out, dtype=np.float32)


# revision 23
# speedup vs baseline: 1.4113x; 1.4113x over previous
"""DeepSeek-style MLA decode attention (batch=8, 128 heads, cache 512) on 8 NeuronCores.

Sharding: tensor-parallel over heads (16 heads/core).
 - q LoRA path sharded over the rank dim (Wq_down cols / Wq_up rows); partial
   q summed+scattered to head owners with a ReduceScatter.
 - Wkv_down replicated (c_kv computed fully on every core).
 - k_cache passed host-pretransposed as [h, b, d, keys]; v_cache as [h, b, keys, d].
 - o_proj input rows sharded by head; partial outputs ReduceScattered over the
   batch dim (core b returns batch b's final row).

Note: the reference's "new token" softmax is over a length-1 axis (== 1.0), so
k_new/Wk_up are dead and the new-token contribution is simply + v_new.
"""

import numpy as np

import concourse.bass as bass
import concourse.mybir as mybir
import concourse.tile as tile
from concourse import bacc
from concourse import bass_utils
from concourse.masks import make_identity

NC_ = 8                      # cores
B = 8                        # batch
H = 128                      # total heads
HP = H // NC_                # 16 heads per core
D = 128                      # head dim
L = 512                      # cache len
HID = 7168
QL = 1536
QLP = QL // NC_              # 192
KVL = 512
NH = HP * D                  # 2048 per-core head cols
SCALE = 1.0 / float(np.sqrt(D))
F32 = mybir.dt.float32
# float32r: single-pass fp32 matmul mode (1 cycle/row at N>=256 vs 4 for
# two-pass fp32). Slightly reduced multiply precision; flip off if the
# accuracy gate complains.
USE_F32R = True


F32R = mybir.dt.float32r
MMD = F32R if USE_F32R else F32  # dtype for matmul-operand tiles


def _rb(ap):
    """Bitcast a DRAM f32 source AP for DMA into a float32r tile."""
    return ap.bitcast(F32R) if USE_F32R else ap


def build_nc():
    nc = bacc.Bacc(
        "TRN2",
        target_bir_lowering=False,
        debug=False,
        enable_asserts=True,
        num_devices=NC_,
    )
    xt = nc.dram_tensor("xt", [HID, B], F32, kind="ExternalInput").ap()
    w_down = nc.dram_tensor("w_down", [HID, QLP + KVL], F32, kind="ExternalInput").ap()
    wq_up = nc.dram_tensor("wq_up", [QLP, H * D], F32, kind="ExternalInput").ap()
    wv_up = nc.dram_tensor("wv_up", [KVL, NH], F32, kind="ExternalInput").ap()
    kt = nc.dram_tensor("kt", [HP, B, D, L], F32, kind="ExternalInput").ap()
    v = nc.dram_tensor("v", [HP, B, L, D], F32, kind="ExternalInput").ap()
    wo = nc.dram_tensor("wo", [NH, HID], F32, kind="ExternalInput").ap()
    o = nc.dram_tensor("o", [1, HID], F32, kind="ExternalOutput").ap()

    rg = [list(range(NC_))]

    with tile.TileContext(nc) as tc:
        with (
            tc.tile_pool(name="const", bufs=1) as constp,
            tc.tile_pool(name="sbuf", bufs=1) as sb,
            tc.tile_pool(name="stage", bufs=2) as stg,
            tc.tile_pool(name="wdown", bufs=3) as wdp,
            tc.tile_pool(name="wqup", bufs=2) as wqp,
            tc.tile_pool(name="ktp", bufs=3) as ktp,
            tc.tile_pool(name="vp", bufs=3) as vp,
            tc.tile_pool(name="wop", bufs=3) as wop,
            tc.tile_pool(name="psbank", bufs=6, space="PSUM") as psbank,
            tc.tile_pool(name="pstr", bufs=2, space="PSUM") as pstr,
            tc.tile_pool(name="dram", bufs=1, space="DRAM") as dram,
        ):
            ident = constp.tile([128, 128], F32)
            make_identity(nc, ident[:])
            id8 = ident[0:8, 0:8]
            # uint8 one-hot columns for CopyPredicated masks (must be int dtype)
            identu8 = constp.tile([128, 128], mybir.dt.uint8, tag="identu8")
            nc.vector.tensor_copy(identu8[:], ident[:])

            # ---------------- q path: cdown = x @ [Wq_down_c | Wkv_down] ----------------
            xt_sb = constp.tile([128, 56 * B], MMD, tag="xt")
            nc.sync.dma_start(
                out=xt_sb[:].rearrange("p (c b) -> p c b", c=56),
                in_=_rb(xt).rearrange("(c p) b -> p c b", p=128),
            )
            ps_cd0 = psbank.tile([8, 512], F32, tag="bank")
            ps_cd1 = psbank.tile([8, 512], F32, tag="bank")
            for i in range(56):
                wd_t = wdp.tile([128, QLP + KVL], MMD, tag="wd")
                nc.sync.dma_start(out=wd_t[:], in_=_rb(w_down)[i * 128:(i + 1) * 128, :])
                lhs = xt_sb[:, i * B:(i + 1) * B]
                nc.tensor.matmul(
                    ps_cd0[:8, 0:512], (lhs), (wd_t[:, 0:512]),
                    start=(i == 0), stop=(i == 55),
                )
                nc.tensor.matmul(
                    ps_cd1[:8, 0:192], lhs, wd_t[:, 512:704],
                    start=(i == 0), stop=(i == 55),
                )
            cdown = sb.tile([8, QLP + KVL], F32, tag="cdown")
            nc.vector.tensor_copy(cdown[:, 0:512], ps_cd0[:8, 0:512])
            nc.vector.tensor_copy(cdown[:, 512:704], ps_cd1[:8, 0:192])

            # transposes: cqT [192, 8] (2 chunks), ckvT [512dims -> 4 chunks of [128, 8]]
            ps_cqT = pstr.tile([128, 128], F32, tag="tr")
            nc.tensor.transpose(ps_cqT[0:128, 0:8], cdown[:, 0:128], id8)
            nc.tensor.transpose(ps_cqT[0:64, 8:16], cdown[:, 128:192], id8)
            ps_ckvT = pstr.tile([128, 128], F32, tag="tr")
            for j in range(4):
                nc.tensor.transpose(
                    ps_ckvT[0:128, j * 8:(j + 1) * 8],
                    cdown[:, QLP + j * 128:QLP + (j + 1) * 128],
                    id8,
                )
            cqT = sb.tile([128, 16], MMD, tag="cqT")
            nc.vector.tensor_copy(cqT[:, 0:8], ps_cqT[:, 0:8])
            nc.vector.tensor_copy(cqT[0:64, 8:16], ps_cqT[0:64, 8:16])
            ckvT = sb.tile([128, 32], MMD, tag="ckvT")
            nc.vector.tensor_copy(ckvT[:, 0:32], ps_ckvT[:, 0:32])

            # ---------------- q_part = cq @ Wq_up_c  (8, 16384) ----------------
            # The 8 col-groups of 2048 are exactly the per-core head groups g;
            # store each to q_bounce[g] for the ReduceScatter.
            q_bounce = dram.tile([NC_ * B, NH], F32, tag="qb")
            for g in range(NC_):
                wqA = wqp.tile([128, 2048], MMD, tag="wqA")
                nc.sync.dma_start(
                    out=wqA[:], in_=_rb(wq_up)[0:128, g * 2048:(g + 1) * 2048]
                )
                wqB = wqp.tile([64, 2048], MMD, tag="wqB")
                nc.sync.dma_start(
                    out=wqB[:], in_=_rb(wq_up)[128:192, g * 2048:(g + 1) * 2048]
                )
                qstage = stg.tile([8, NH], F32, tag="qstage")
                for j in range(4):
                    ps_q = psbank.tile([8, 512], F32, tag="bank")
                    nc.tensor.matmul(
                        ps_q[:8, :], (cqT[:, 0:8]), (wqA[:, j * 512:(j + 1) * 512]),
                        start=True, stop=False,
                    )
                    nc.tensor.matmul(
                        ps_q[:8, :], (cqT[0:64, 8:16]), (wqB[:, j * 512:(j + 1) * 512]),
                        start=False, stop=True,
                    )
                    nc.vector.tensor_copy(
                        qstage[:, j * 512:(j + 1) * 512], ps_q[:8, :]
                    )
                nc.sync.dma_start(
                    out=q_bounce[g * B:(g + 1) * B, :], in_=qstage[:]
                )
            q_rs = dram.tile([B, NH], F32, tag="qrs")
            nc.gpsimd.collective_compute(
                "ReduceScatter",
                mybir.AluOpType.add,
                replica_groups=rg,
                ins=[q_bounce.opt()],
                outs=[q_rs.opt()],
            )
            qown = sb.tile([8, NH], F32, tag="qown")
            nc.sync.dma_start(out=qown[:], in_=q_rs[:])

            # ---------------- v_new = ckv @ Wv_up_c  (8, 2048) ----------------
            wvup = constp.tile([128, 4 * NH], MMD, tag="wvup")
            nc.sync.dma_start(
                out=wvup[:].rearrange("p (c n) -> p c n", c=4),
                in_=_rb(wv_up).rearrange("(c p) n -> p c n", p=128),
            )
            vnew = sb.tile([8, NH], F32, tag="vnew")
            for j in range(4):
                ps_v = psbank.tile([8, 512], F32, tag="bank")
                for cc in range(4):
                    nc.tensor.matmul(
                        ps_v[:8, :],
                        (ckvT[:, cc * 8:(cc + 1) * 8]),
                        (wvup[:, cc * NH + j * 512:cc * NH + (j + 1) * 512]),
                        start=(cc == 0), stop=(cc == 3),
                    )
                nc.vector.tensor_copy(vnew[:, j * 512:(j + 1) * 512], ps_v[:8, :])

            # qT [128 d, hb] via 16 transposes
            ps_qT = pstr.tile([128, 128], F32, tag="tr")
            for h in range(HP):
                nc.tensor.transpose(
                    ps_qT[0:128, h * 8:(h + 1) * 8],
                    qown[:, h * D:(h + 1) * D],
                    id8,
                )
            qT = sb.tile([128, 128], MMD, tag="qT")
            nc.vector.tensor_copy(qT[:], ps_qT[:])

            # ---------------- phase A: scores over k cache ----------------
            # lhsT = qT (all 128 hb) stationary; rhs = kT tile (moving, N=512).
            # Out row hb of each full-bank product is the valid score row;
            # extract it with a partition-aligned copy.
            kt_flat = _rb(kt).rearrange("h b d k -> (h b) d k")
            scores = sb.tile([128, 512], F32, tag="scores")
            for t in range(32):
                kt_t = ktp.tile([128, 2048], MMD, tag="kt")
                nc.sync.dma_start(
                    out=kt_t[:].rearrange("d (t k) -> d t k", t=4),
                    in_=kt_flat[4 * t:4 * t + 4].rearrange("t d k -> d t k"),
                )
                for u in range(4):
                    hb = 4 * t + u
                    ps_s = psbank.tile([128, 512], F32, tag="bank")
                    nc.tensor.matmul(
                        ps_s[:],
                        (qT[:]),
                        (kt_t[:, u * 512:(u + 1) * 512]),
                        start=True, stop=True,
                    )
                    # write only row hb (engines can't address partition hb
                    # directly: start partition must be 0/32/64/96)
                    nc.vector.copy_predicated(
                        scores[:],
                        identu8[:, hb:hb + 1].broadcast_to((128, 512)),
                        ps_s[:],
                    )

            probs = sb.tile([128, 512], F32, tag="probs")
            denom = sb.tile([128, 1], F32, tag="denom")
            nc.scalar.activation(
                probs[:], scores[:], mybir.ActivationFunctionType.Exp,
                scale=SCALE, accum_out=denom[:],
            )
            recip = sb.tile([128, 1], F32, tag="recip")
            nc.vector.reciprocal(recip[:], denom[:])
            probsn = sb.tile([128, 512], F32, tag="probsn")
            nc.vector.tensor_scalar_mul(probsn[:], probs[:], recip[:])

            ps_pT = psbank.tile([128, 512], F32, tag="bank")
            for cc in range(4):
                nc.tensor.transpose(
                    ps_pT[:, cc * 128:(cc + 1) * 128],
                    probsn[:, cc * 128:(cc + 1) * 128],
                    ident[:],
                )
            probsT = sb.tile([128, 512], MMD, tag="probsT")
            nc.vector.tensor_copy(probsT[:], ps_pT[:])

            # ---------------- phase B: attn rows = probs @ V ----------------
            # Per group of 4 hb: lhsT = probsT chunk c (all hb), rhs packs the
            # 4 hb's V chunk c side by side; accumulate over c, then extract
            # row 4g+u from column block u.
            v_flat = _rb(v).rearrange("h b l d -> (h b) l d")
            attn = sb.tile([128, 128], F32, tag="attn")
            for g in range(32):
                v_t = vp.tile([128, 2048], MMD, tag="v")
                for cc in range(4):
                    nc.sync.dma_start(
                        out=v_t[:, cc * 512:(cc + 1) * 512].rearrange(
                            "k (t d) -> k t d", t=4
                        ),
                        in_=v_flat[
                            4 * g:4 * g + 4, cc * 128:(cc + 1) * 128, :
                        ].rearrange("t k d -> k t d"),
                    )
                ps_a = psbank.tile([128, 512], F32, tag="bank")
                for cc in range(4):
                    nc.tensor.matmul(
                        ps_a[:],
                        (probsT[:, cc * 128:(cc + 1) * 128]),
                        (v_t[:, cc * 512:(cc + 1) * 512]),
                        start=(cc == 0), stop=(cc == 3),
                    )
                for u in range(4):
                    hb = 4 * g + u
                    nc.vector.copy_predicated(
                        attn[:],
                        identu8[:, hb:hb + 1].broadcast_to((128, 128)),
                        ps_a[:, u * 128:(u + 1) * 128],
                    )

            # attnT = attn^T + v_new^T
            ps_vT = pstr.tile([128, 128], F32, tag="tr")
            for h in range(HP):
                nc.tensor.transpose(
                    ps_vT[0:128, h * 8:(h + 1) * 8],
                    vnew[:, h * D:(h + 1) * D],
                    id8,
                )
            vnewT = sb.tile([128, 128], F32, tag="vnewT")
            nc.vector.tensor_copy(vnewT[:], ps_vT[:])
            ps_aT = pstr.tile([128, 128], F32, tag="tr")
            nc.tensor.transpose(ps_aT[:], attn[:], ident[:])
            attnT = sb.tile([128, 128], MMD, tag="attnT")
            nc.vector.tensor_add(attnT[:], ps_aT[:], vnewT[:])

            # ---------------- phase C: o_part = attn^T @ Wo_c ----------------
            # Rounds of up to 6 n-chunks so the accumulators fit in the bank
            # pool; Wo streams as per-head row blocks (large contiguous runs).
            o_bounce = dram.tile([B, HID], F32, tag="ob")
            for n0, n1 in ((0, 6), (6, 12), (12, 14)):
                nn = n1 - n0
                ps_os = [
                    psbank.tile([8, 512], F32, tag="bank", name=f"ps_o{n0}_{i}")
                    for i in range(nn)
                ]
                for h in range(HP):
                    wo_t = wop.tile([128, 3072], MMD, tag="wo")
                    nc.sync.dma_start(
                        out=wo_t[:, 0:nn * 512],
                        in_=_rb(wo)[h * D:(h + 1) * D, n0 * 512:n1 * 512],
                    )
                    for i in range(nn):
                        nc.tensor.matmul(
                            ps_os[i][:8, :],
                            (attnT[:, h * 8:(h + 1) * 8]),
                            (wo_t[:, i * 512:(i + 1) * 512]),
                            start=(h == 0), stop=(h == HP - 1),
                        )
                for i in range(nn):
                    ostage = stg.tile([8, 512], F32, tag="ostage")
                    nc.vector.tensor_copy(ostage[:], ps_os[i][:8, :])
                    nc.sync.dma_start(
                        out=o_bounce[:, (n0 + i) * 512:(n0 + i + 1) * 512],
                        in_=ostage[:],
                    )

            o_rs = dram.tile([1, HID], F32, tag="ors")
            nc.gpsimd.collective_compute(
                "ReduceScatter",
                mybir.AluOpType.add,
                replica_groups=rg,
                ins=[o_bounce.opt()],
                outs=[o_rs.opt()],
            )
            nc.sync.dma_start(out=o[:], in_=o_rs[:])

    nc.compile()
    return nc


_NC_CACHE = None


def _get_nc():
    global _NC_CACHE
    if _NC_CACHE is None:
        _NC_CACHE = build_nc()
    return _NC_CACHE


def make_in_maps(x, k_cache, v_cache, Wq_down, Wq_up, Wkv_down, Wv_up, Wo):
    x2 = np.ascontiguousarray(np.asarray(x, dtype=np.float32).reshape(B, HID).T)
    in_maps = []
    for c in range(NC_):
        hs = slice(c * HP, (c + 1) * HP)
        w_down_c = np.ascontiguousarray(
            np.concatenate(
                [Wq_down[:, c * QLP:(c + 1) * QLP], Wkv_down], axis=1
            ).astype(np.float32)
        )
        wq_up_c = np.ascontiguousarray(Wq_up[c * QLP:(c + 1) * QLP, :], dtype=np.float32)
        wv_up_c = np.ascontiguousarray(
            Wv_up[:, c * HP * D:(c + 1) * HP * D], dtype=np.float32
        )
        wo_c = np.ascontiguousarray(
            Wo[c * HP * D:(c + 1) * HP * D, :], dtype=np.float32
        )
        kt_c = np.ascontiguousarray(
            np.asarray(k_cache, dtype=np.float32)[:, hs].transpose(1, 0, 3, 2)
        )
        v_c = np.ascontiguousarray(
            np.asarray(v_cache, dtype=np.float32)[:, hs].transpose(1, 0, 2, 3)
        )
        in_maps.append(
            {
                "xt": x2,
                "w_down": w_down_c,
                "wq_up": wq_up_c,
                "wv_up": wv_up_c,
                "kt": kt_c,
                "v": v_c,
                "wo": wo_c,
            }
        )
    return in_maps


def kernel(x, k_cache, v_cache, Wq_down, Wq_up, Wkv_down, Wk_up, Wv_up, Wo, **_):
    x = np.asarray(x, dtype=np.float32)
    in_maps = make_in_maps(
        x, np.asarray(k_cache), np.asarray(v_cache),
        np.asarray(Wq_down, dtype=np.float32), np.asarray(Wq_up, dtype=np.float32),
        np.asarray(Wkv_down, dtype=np.float32), np.asarray(Wv_up, dtype=np.float32),
        np.asarray(Wo, dtype=np.float32),
    )
    nc = _get_nc()
    res = bass_utils.run_bass_kernel_spmd(nc, in_maps, core_ids=list(range(NC_)))
    out = np.stack([res.results[b]["o"] for b in range(B)], axis=0)  # (8, 1, 7168)
    return np.ascontiguousarray(out, dtype=np.float32)


# revision 26
# speedup vs baseline: 1.4918x; 1.0570x over previous
"""DeepSeek-style MLA decode attention (batch=8, 128 heads, cache 512) on 8 NeuronCores.

Sharding: tensor-parallel over heads (16 heads/core).
 - q LoRA path sharded over the rank dim (Wq_down cols / Wq_up rows); partial
   q summed+scattered to head owners with a ReduceScatter.
 - Wkv_down replicated (c_kv computed fully on every core).
 - k_cache passed host-pretransposed as [h, b, d, keys]; v_cache as [h, b, keys, d].
 - o_proj input rows sharded by head; partial outputs ReduceScattered over the
   batch dim (core b returns batch b's final row).

Note: the reference's "new token" softmax is over a length-1 axis (== 1.0), so
k_new/Wk_up are dead and the new-token contribution is simply + v_new.
"""

import numpy as np

import concourse.bass as bass
import concourse.mybir as mybir
import concourse.tile as tile
from concourse import bacc
from concourse import bass_utils
from concourse.masks import make_identity

NC_ = 8                      # cores
B = 8                        # batch
H = 128                      # total heads
HP = H // NC_                # 16 heads per core
D = 128                      # head dim
L = 512                      # cache len
HID = 7168
QL = 1536
QLP = QL // NC_              # 192
KVL = 512
NH = HP * D                  # 2048 per-core head cols
SCALE = 1.0 / float(np.sqrt(D))
F32 = mybir.dt.float32
# float32r: single-pass fp32 matmul mode (1 cycle/row at N>=256 vs 4 for
# two-pass fp32). Slightly reduced multiply precision; flip off if the
# accuracy gate complains.
USE_F32R = True


F32R = mybir.dt.float32r
MMD = F32R if USE_F32R else F32  # dtype for matmul-operand tiles


def _rb(ap):
    """Bitcast a DRAM f32 source AP for DMA into a float32r tile."""
    return ap.bitcast(F32R) if USE_F32R else ap


def build_nc():
    nc = bacc.Bacc(
        "TRN2",
        target_bir_lowering=False,
        debug=False,
        enable_asserts=True,
        num_devices=NC_,
    )
    xt = nc.dram_tensor("xt", [HID, B], F32, kind="ExternalInput").ap()
    w_down = nc.dram_tensor("w_down", [HID, QLP + KVL], F32, kind="ExternalInput").ap()
    wq_up = nc.dram_tensor("wq_up", [QLP, H * D], F32, kind="ExternalInput").ap()
    wv_up = nc.dram_tensor("wv_up", [KVL, NH], F32, kind="ExternalInput").ap()
    kt = nc.dram_tensor("kt", [32, 128, 2048], F32, kind="ExternalInput").ap()
    v = nc.dram_tensor("v", [32, 128, 2048], F32, kind="ExternalInput").ap()
    wo = nc.dram_tensor("wo", [NH, HID], F32, kind="ExternalInput").ap()
    o = nc.dram_tensor("o", [1, HID], F32, kind="ExternalOutput").ap()

    rg = [list(range(NC_))]

    with tile.TileContext(nc) as tc:
        with (
            tc.tile_pool(name="const", bufs=1) as constp,
            tc.tile_pool(name="sbuf", bufs=1) as sb,
            tc.tile_pool(name="stage", bufs=2) as stg,
            tc.tile_pool(name="wdown", bufs=3) as wdp,
            tc.tile_pool(name="wqup", bufs=2) as wqp,
            tc.tile_pool(name="ktp", bufs=3) as ktp,
            tc.tile_pool(name="vp", bufs=3) as vp,
            tc.tile_pool(name="wop", bufs=3) as wop,
            tc.tile_pool(name="psbank", bufs=6, space="PSUM") as psbank,
            tc.tile_pool(name="pstr", bufs=2, space="PSUM") as pstr,
            tc.tile_pool(name="dram", bufs=1, space="DRAM") as dram,
        ):
            ident = constp.tile([128, 128], F32)
            make_identity(nc, ident[:])
            id8 = ident[0:8, 0:8]
            # uint8 one-hot columns for CopyPredicated masks (must be int dtype)
            identu8 = constp.tile([128, 128], mybir.dt.uint8, tag="identu8")
            nc.vector.tensor_copy(identu8[:], ident[:])

            # ---------------- q path: cdown = x @ [Wq_down_c | Wkv_down] ----------------
            xt_sb = constp.tile([128, 56 * B], MMD, tag="xt")
            nc.sync.dma_start(
                out=xt_sb[:].rearrange("p (c b) -> p c b", c=56),
                in_=_rb(xt).rearrange("(c p) b -> p c b", p=128),
            )
            ps_cd0 = psbank.tile([8, 512], F32, tag="bank")
            ps_cd1 = psbank.tile([8, 512], F32, tag="bank")
            for i in range(56):
                wd_t = wdp.tile([128, QLP + KVL], MMD, tag="wd")
                nc.sync.dma_start(out=wd_t[:], in_=_rb(w_down)[i * 128:(i + 1) * 128, :])
                lhs = xt_sb[:, i * B:(i + 1) * B]
                nc.tensor.matmul(
                    ps_cd0[:8, 0:512], (lhs), (wd_t[:, 0:512]),
                    start=(i == 0), stop=(i == 55),
                )
                nc.tensor.matmul(
                    ps_cd1[:8, 0:192], lhs, wd_t[:, 512:704],
                    start=(i == 0), stop=(i == 55),
                )
            cdown = sb.tile([8, QLP + KVL], F32, tag="cdown")
            nc.vector.tensor_copy(cdown[:, 0:512], ps_cd0[:8, 0:512])
            nc.vector.tensor_copy(cdown[:, 512:704], ps_cd1[:8, 0:192])

            # transposes: cqT [192, 8] (2 chunks), ckvT [512dims -> 4 chunks of [128, 8]]
            ps_cqT = pstr.tile([128, 128], F32, tag="tr")
            nc.tensor.transpose(ps_cqT[0:128, 0:8], cdown[:, 0:128], id8)
            nc.tensor.transpose(ps_cqT[0:64, 8:16], cdown[:, 128:192], id8)
            ps_ckvT = pstr.tile([128, 128], F32, tag="tr")
            for j in range(4):
                nc.tensor.transpose(
                    ps_ckvT[0:128, j * 8:(j + 1) * 8],
                    cdown[:, QLP + j * 128:QLP + (j + 1) * 128],
                    id8,
                )
            cqT = sb.tile([128, 16], MMD, tag="cqT")
            nc.vector.tensor_copy(cqT[:, 0:8], ps_cqT[:, 0:8])
            nc.vector.tensor_copy(cqT[0:64, 8:16], ps_cqT[0:64, 8:16])
            ckvT = sb.tile([128, 32], MMD, tag="ckvT")
            nc.vector.tensor_copy(ckvT[:, 0:32], ps_ckvT[:, 0:32])

            # ---------------- q_part = cq @ Wq_up_c  (8, 16384) ----------------
            # The 8 col-groups of 2048 are exactly the per-core head groups g;
            # store each to q_bounce[g] for the ReduceScatter.
            q_bounce = dram.tile([NC_ * B, NH], F32, tag="qb")
            for g in range(NC_):
                wqA = wqp.tile([128, 2048], MMD, tag="wqA")
                nc.sync.dma_start(
                    out=wqA[:], in_=_rb(wq_up)[0:128, g * 2048:(g + 1) * 2048]
                )
                wqB = wqp.tile([64, 2048], MMD, tag="wqB")
                nc.sync.dma_start(
                    out=wqB[:], in_=_rb(wq_up)[128:192, g * 2048:(g + 1) * 2048]
                )
                qstage = stg.tile([8, NH], F32, tag="qstage")
                for j in range(4):
                    ps_q = psbank.tile([8, 512], F32, tag="bank")
                    nc.tensor.matmul(
                        ps_q[:8, :], (cqT[:, 0:8]), (wqA[:, j * 512:(j + 1) * 512]),
                        start=True, stop=False,
                    )
                    nc.tensor.matmul(
                        ps_q[:8, :], (cqT[0:64, 8:16]), (wqB[:, j * 512:(j + 1) * 512]),
                        start=False, stop=True,
                    )
                    nc.vector.tensor_copy(
                        qstage[:, j * 512:(j + 1) * 512], ps_q[:8, :]
                    )
                nc.sync.dma_start(
                    out=q_bounce[g * B:(g + 1) * B, :], in_=qstage[:]
                )
            q_rs = dram.tile([B, NH], F32, tag="qrs")
            nc.gpsimd.collective_compute(
                "ReduceScatter",
                mybir.AluOpType.add,
                replica_groups=rg,
                ins=[q_bounce.opt()],
                outs=[q_rs.opt()],
            )
            qown = sb.tile([8, NH], F32, tag="qown")
            nc.sync.dma_start(out=qown[:], in_=q_rs[:])

            # ---------------- v_new = ckv @ Wv_up_c  (8, 2048) ----------------
            wvup = constp.tile([128, 4 * NH], MMD, tag="wvup")
            nc.sync.dma_start(
                out=wvup[:].rearrange("p (c n) -> p c n", c=4),
                in_=_rb(wv_up).rearrange("(c p) n -> p c n", p=128),
            )
            vnew = sb.tile([8, NH], F32, tag="vnew")
            for j in range(4):
                ps_v = psbank.tile([8, 512], F32, tag="bank")
                for cc in range(4):
                    nc.tensor.matmul(
                        ps_v[:8, :],
                        (ckvT[:, cc * 8:(cc + 1) * 8]),
                        (wvup[:, cc * NH + j * 512:cc * NH + (j + 1) * 512]),
                        start=(cc == 0), stop=(cc == 3),
                    )
                nc.vector.tensor_copy(vnew[:, j * 512:(j + 1) * 512], ps_v[:8, :])

            # qT [128 d, hb] via 16 transposes
            ps_qT = pstr.tile([128, 128], F32, tag="tr")
            for h in range(HP):
                nc.tensor.transpose(
                    ps_qT[0:128, h * 8:(h + 1) * 8],
                    qown[:, h * D:(h + 1) * D],
                    id8,
                )
            qT = sb.tile([128, 128], MMD, tag="qT")
            nc.vector.tensor_copy(qT[:], ps_qT[:])

            # ---------------- phase A: scores over k cache ----------------
            # lhsT = qT (all 128 hb) stationary; rhs = kT tile (moving, N=512).
            # Out row hb of each full-bank product is the valid score row;
            # extract it with a partition-aligned copy.
            scores = sb.tile([128, 512], F32, tag="scores")
            for t in range(32):
                kt_t = ktp.tile([128, 2048], MMD, tag="kt")
                nc.sync.dma_start(out=kt_t[:], in_=_rb(kt)[t])
                for u in range(4):
                    hb = 4 * t + u
                    ps_s = psbank.tile([128, 512], F32, tag="bank")
                    nc.tensor.matmul(
                        ps_s[:],
                        (qT[:]),
                        (kt_t[:, u * 512:(u + 1) * 512]),
                        start=True, stop=True,
                    )
                    # write only row hb (engines can't address partition hb
                    # directly: start partition must be 0/32/64/96)
                    nc.vector.copy_predicated(
                        scores[:],
                        identu8[:, hb:hb + 1].broadcast_to((128, 512)),
                        ps_s[:],
                    )

            probs = sb.tile([128, 512], F32, tag="probs")
            denom = sb.tile([128, 1], F32, tag="denom")
            nc.scalar.activation(
                probs[:], scores[:], mybir.ActivationFunctionType.Exp,
                scale=SCALE, accum_out=denom[:],
            )
            recip = sb.tile([128, 1], F32, tag="recip")
            nc.vector.reciprocal(recip[:], denom[:])
            probsn = sb.tile([128, 512], F32, tag="probsn")
            nc.vector.tensor_scalar_mul(probsn[:], probs[:], recip[:])

            ps_pT = psbank.tile([128, 512], F32, tag="bank")
            for cc in range(4):
                nc.tensor.transpose(
                    ps_pT[:, cc * 128:(cc + 1) * 128],
                    probsn[:, cc * 128:(cc + 1) * 128],
                    ident[:],
                )
            probsT = sb.tile([128, 512], MMD, tag="probsT")
            nc.vector.tensor_copy(probsT[:], ps_pT[:])

            # ---------------- phase B: attn rows = probs @ V ----------------
            # Per group of 4 hb: lhsT = probsT chunk c (all hb), rhs packs the
            # 4 hb's V chunk c side by side; accumulate over c, then extract
            # row 4g+u from column block u.
            attn = sb.tile([128, 128], F32, tag="attn")
            for g in range(32):
                v_t = vp.tile([128, 2048], MMD, tag="v")
                nc.sync.dma_start(out=v_t[:], in_=_rb(v)[g])
                ps_a = psbank.tile([128, 512], F32, tag="bank")
                for cc in range(4):
                    nc.tensor.matmul(
                        ps_a[:],
                        (probsT[:, cc * 128:(cc + 1) * 128]),
                        (v_t[:, cc * 512:(cc + 1) * 512]),
                        start=(cc == 0), stop=(cc == 3),
                    )
                for u in range(4):
                    hb = 4 * g + u
                    nc.vector.copy_predicated(
                        attn[:],
                        identu8[:, hb:hb + 1].broadcast_to((128, 128)),
                        ps_a[:, u * 128:(u + 1) * 128],
                    )

            # attnT = attn^T + v_new^T
            ps_vT = pstr.tile([128, 128], F32, tag="tr")
            for h in range(HP):
                nc.tensor.transpose(
                    ps_vT[0:128, h * 8:(h + 1) * 8],
                    vnew[:, h * D:(h + 1) * D],
                    id8,
                )
            vnewT = sb.tile([128, 128], F32, tag="vnewT")
            nc.vector.tensor_copy(vnewT[:], ps_vT[:])
            ps_aT = pstr.tile([128, 128], F32, tag="tr")
            nc.tensor.transpose(ps_aT[:], attn[:], ident[:])
            attnT = sb.tile([128, 128], MMD, tag="attnT")
            nc.vector.tensor_add(attnT[:], ps_aT[:], vnewT[:])

            # ---------------- phase C: o_part = attn^T @ Wo_c ----------------
            # Rounds of up to 6 n-chunks so the accumulators fit in the bank
            # pool; Wo streams as per-head row blocks (large contiguous runs).
            o_bounce = dram.tile([B, HID], F32, tag="ob")
            for n0, n1 in ((0, 6), (6, 12), (12, 14)):
                nn = n1 - n0
                ps_os = [
                    psbank.tile([8, 512], F32, tag="bank", name=f"ps_o{n0}_{i}")
                    for i in range(nn)
                ]
                for h in range(HP):
                    wo_t = wop.tile([128, 3072], MMD, tag="wo")
                    nc.sync.dma_start(
                        out=wo_t[:, 0:nn * 512],
                        in_=_rb(wo)[h * D:(h + 1) * D, n0 * 512:n1 * 512],
                    )
                    for i in range(nn):
                        nc.tensor.matmul(
                            ps_os[i][:8, :],
                            (attnT[:, h * 8:(h + 1) * 8]),
                            (wo_t[:, i * 512:(i + 1) * 512]),
                            start=(h == 0), stop=(h == HP - 1),
                        )
                for i in range(nn):
                    ostage = stg.tile([8, 512], F32, tag="ostage")
                    nc.vector.tensor_copy(ostage[:], ps_os[i][:8, :])
                    nc.sync.dma_start(
                        out=o_bounce[:, (n0 + i) * 512:(n0 + i + 1) * 512],
                        in_=ostage[:],
                    )

            o_rs = dram.tile([1, HID], F32, tag="ors")
            nc.gpsimd.collective_compute(
                "ReduceScatter",
                mybir.AluOpType.add,
                replica_groups=rg,
                ins=[o_bounce.opt()],
                outs=[o_rs.opt()],
            )
            nc.sync.dma_start(out=o[:], in_=o_rs[:])

    nc.compile()
    return nc


_NC_CACHE = None


def _get_nc():
    global _NC_CACHE
    if _NC_CACHE is None:
        _NC_CACHE = build_nc()
    return _NC_CACHE


def make_in_maps(x, k_cache, v_cache, Wq_down, Wq_up, Wkv_down, Wv_up, Wo):
    x2 = np.ascontiguousarray(np.asarray(x, dtype=np.float32).reshape(B, HID).T)
    in_maps = []
    for c in range(NC_):
        hs = slice(c * HP, (c + 1) * HP)
        w_down_c = np.ascontiguousarray(
            np.concatenate(
                [Wq_down[:, c * QLP:(c + 1) * QLP], Wkv_down], axis=1
            ).astype(np.float32)
        )
        wq_up_c = np.ascontiguousarray(Wq_up[c * QLP:(c + 1) * QLP, :], dtype=np.float32)
        wv_up_c = np.ascontiguousarray(
            Wv_up[:, c * HP * D:(c + 1) * HP * D], dtype=np.float32
        )
        wo_c = np.ascontiguousarray(
            Wo[c * HP * D:(c + 1) * HP * D, :], dtype=np.float32
        )
        # kt tile g holds hb=4g..4g+4 as [128 d, (t, k)]; hb=(h, b) row-major
        kt_c = np.ascontiguousarray(
            np.asarray(k_cache, dtype=np.float32)[:, hs]
            .transpose(1, 0, 3, 2)          # (16, 8, 128, 512) [h, b, d, k]
            .reshape(32, 4, 128, 512)       # [g, t, d, k]
            .transpose(0, 2, 1, 3)          # [g, d, t, k]
            .reshape(32, 128, 2048)
        )
        # v tile g holds hb=4g..4g+4 as [128 k, (c, t, d)]
        v_c = np.ascontiguousarray(
            np.asarray(v_cache, dtype=np.float32)[:, hs]
            .transpose(1, 0, 2, 3)          # (16, 8, 512, 128) [h, b, l, d]
            .reshape(32, 4, 4, 128, 128)    # [g, t, c, k, d]
            .transpose(0, 3, 2, 1, 4)       # [g, k, c, t, d]
            .reshape(32, 128, 2048)
        )
        in_maps.append(
            {
                "xt": x2,
                "w_down": w_down_c,
                "wq_up": wq_up_c,
                "wv_up": wv_up_c,
                "kt": kt_c,
                "v": v_c,
                "wo": wo_c,
            }
        )
    return in_maps


def kernel(x, k_cache, v_cache, Wq_down, Wq_up, Wkv_down, Wk_up, Wv_up, Wo, **_):
    x = np.asarray(x, dtype=np.float32)
    in_maps = make_in_maps(
        x, np.asarray(k_cache), np.asarray(v_cache),
        np.asarray(Wq_down, dtype=np.float32), np.asarray(Wq_up, dtype=np.float32),
        np.asarray(Wkv_down, dtype=np.float32), np.asarray(Wv_up, dtype=np.float32),
        np.asarray(Wo, dtype=np.float32),
    )
    nc = _get_nc()
    res = bass_utils.run_bass_kernel_spmd(nc, in_maps, core_ids=list(range(NC_)))
    out = np.stack([res.results[b]["o"] for b in range(B)], axis=0)  # (8, 1, 7168)
    return np.ascontiguousarray(out, dtype=np.float32)
